# revision 4
# baseline (speedup 1.0000x reference)
"""Trainium2 Bass kernel for the Kuramoto layer (nn_KuramotoLayer_60224031425028).

theta_{t+1} = theta_t + dt*(omega + c*(K@s) - s*(K@c)),  s=sin(theta), c=cos(theta)
N=4096, 50 steps, K symmetric (not relied upon - host feeds explicit K^T slices).

Strategy (8 cores, SPMD one NEFF):
  - Row-shard: core r owns rows [512r, 512r+512). Host feeds kt_r = K[rows_r,:].T
    ([4096, 512] f32, 8MB) which stays resident in SBUF for all 50 steps.
  - Per step, per core: out[2,512] = sum_b sc[b].T @ ktile_b with the [s;c]
    pair as the 2-column stationary operand and K streaming as the moving
    operand (f32r -> 1 cycle/row), accumulated over 32 contraction blocks
    in one PSUM bank.
  - State kept partition-major [128,4]; sin/cos on ACT (range-reduced to
    [-pi,pi] on DVE via magic-number rounding); coupling arithmetic on DVE
    after 4 small PE transposes of the [2,512] matvec result.
  - Cross-core exchange of the new sin/cos slices each step via an
    AllGather collective through DRAM bounce buffers.
"""

import os
import numpy as np

N = 4096
NCORES = 8
LOCAL = N // NCORES          # 512
NSUB = LOCAL // 128          # 4
NBLK = N // 128              # 32
N_STEPS = 50
DT = 0.01
INV_2PI = float(np.float32(1.0 / (2.0 * np.pi)))
TWO_PI = float(np.float32(2.0 * np.pi))
MAGIC = float(np.float32(1.5 * 2 ** 23))
HALF_PI = float(np.float32(np.pi / 2.0))
# clamp bound strictly below float64 pi so the ACT table range check passes
PI_LO = float(np.nextafter(np.float32(np.pi), np.float32(0.0)))


def build_nc(n_steps=N_STEPS):
    import concourse.bass as bass
    import concourse.mybir as mybir
    from contextlib import ExitStack

    F32 = mybir.dt.float32
    F32R = mybir.dt.float32r

    nc = bass.Bass(num_devices=NCORES)

    kt = nc.declare_dram_parameter("kt", [N, LOCAL], F32R, isOutput=False)
    th0 = nc.declare_dram_parameter("th0", [128, NSUB], F32, isOutput=False)
    om = nc.declare_dram_parameter("om", [128, NSUB], F32, isOutput=False)
    eye = nc.declare_dram_parameter("eye", [128, 128], F32, isOutput=False)
    out_ext = nc.declare_dram_parameter("out", [128, NSUB], F32, isOutput=True)

    cc_in = nc.dram_tensor("cc_in", [2 * NSUB, 128], F32)
    cc_out = nc.dram_tensor("cc_out", [2 * NBLK, 128], F32, addr_space="Shared")

    ctx = ExitStack()
    # SBUF
    k_sb = ctx.enter_context(nc.sbuf_tensor("k_sb", [128, NBLK * LOCAL], F32R))
    sc_full = ctx.enter_context(nc.sbuf_tensor("sc_full", [128, 2 * NBLK], F32R))
    sc_loc = ctx.enter_context(nc.sbuf_tensor("sc_loc", [128, 2 * NSUB], F32))
    sc_fm = ctx.enter_context(nc.sbuf_tensor("sc_fm", [2 * NSUB, 128], F32))
    gath_fm = ctx.enter_context(nc.sbuf_tensor("gath_fm", [2 * NBLK, 128], F32))
    th_sb = ctx.enter_context(nc.sbuf_tensor("th_sb", [128, NSUB], F32))
    om_sb = ctx.enter_context(nc.sbuf_tensor("om_sb", [128, NSUB], F32))
    eye_sb = ctx.enter_context(nc.sbuf_tensor("eye_sb", [128, 128], F32))
    kskc = ctx.enter_context(nc.sbuf_tensor("kskc", [2, LOCAL], F32))
    m1 = ctx.enter_context(nc.sbuf_tensor("m1", [128, NSUB], F32))
    m2 = ctx.enter_context(nc.sbuf_tensor("m2", [128, NSUB], F32))
    cpl = ctx.enter_context(nc.sbuf_tensor("cpl", [128, NSUB], F32))
    u1 = ctx.enter_context(nc.sbuf_tensor("u1", [128, NSUB], F32))
    xr_s = ctx.enter_context(nc.sbuf_tensor("xr_s", [128, NSUB], F32))
    xr_c = ctx.enter_context(nc.sbuf_tensor("xr_c", [128, NSUB], F32))
    # PSUM
    ps_mv = ctx.enter_context(nc.psum_tensor("ps_mv", [128, LOCAL], F32))
    ps_T = ctx.enter_context(nc.psum_tensor("ps_T", [128, 2 * NSUB], F32))
    ps_t2 = ctx.enter_context(nc.psum_tensor("ps_t2", [2 * NSUB, 128], F32))
    ps_g = ctx.enter_context(nc.psum_tensor("ps_g", [128, 2 * NBLK], F32))

    # semaphores
    sems = {}
    for name in ["d_ld", "d_k", "s_th", "s_sc", "s_T2", "s_fm", "d_in",
                 "s_cc", "d_out", "s_g", "s_scfull", "s_mv", "s_kskc",
                 "s_T", "d_fin"]:
        sems[name] = ctx.enter_context(nc.semaphore(name))
    d_ld, d_k = sems["d_ld"], sems["d_k"]
    s_th, s_sc, s_T2, s_fm = sems["s_th"], sems["s_sc"], sems["s_T2"], sems["s_fm"]
    d_in, s_cc, d_out = sems["d_in"], sems["s_cc"], sems["d_out"]
    s_g, s_scfull = sems["s_g"], sems["s_scfull"]
    s_mv, s_kskc, s_T = sems["s_mv"], sems["s_kskc"], sems["s_T"]
    d_fin = sems["d_fin"]

    AO = mybir.AluOpType
    SIN = mybir.ActivationFunctionType.Sin

    with nc.Block() as block:

        @block.sync
        def _(sp):
            # input loads
            sp.dma_start(th_sb[:], th0[:]).then_inc(d_ld, 16)
            sp.dma_start(om_sb[:], om[:]).then_inc(d_ld, 16)
            sp.dma_start(eye_sb[:], eye[:]).then_inc(d_ld, 16)
            # K^T: k_sb[p, 512*b + c] = kt[128*b + p, c]
            sp.dma_start(
                k_sb[:].rearrange("p (b c) -> p b c", c=LOCAL),
                kt[:].rearrange("(b p) c -> p b c", p=128),
            ).then_inc(d_k, 16)
            # comm pipeline: item j = sc slices for step j
            for j in range(n_steps):
                sp.wait_ge(s_fm, j + 1)
                if j > 0:
                    sp.wait_ge(s_cc, j)  # WAR: collective j-1 done reading cc_in
                sp.dma_start(cc_in[:], sc_fm[:]).then_inc(d_in, 16)
                sp.wait_ge(s_cc, j + 1)
                if j > 0:
                    sp.wait_ge(s_g, j)  # WAR: PE transpose j-1 done with gath_fm
                sp.dma_start(gath_fm[:], cc_out[:]).then_inc(d_out, 16)
            # final output
            sp.wait_ge(s_th, n_steps + 1)
            sp.dma_start(out_ext[:], th_sb[:]).then_inc(d_fin, 16)

        @block.gpsimd
        def _(gp):
            for j in range(n_steps):
                gp.wait_ge(d_in, 16 * (j + 1))
                if j > 0:
                    gp.wait_ge(d_out, 16 * j)  # WAR: dma2 j-1 done reading cc_out
                gp.collective_compute(
                    "AllGather",
                    AO.bypass,
                    replica_groups=[list(range(NCORES))],
                    ins=[cc_in[:]],
                    outs=[cc_out[:]],
                ).then_inc(s_cc)

        @block.scalar
        def _(act):
            for j in range(n_steps):
                act.wait_ge(s_th, j + 1)
                act.activation(sc_loc[:, 0:2 * NSUB:2], xr_s[:], SIN)
                act.activation(sc_loc[:, 1:2 * NSUB:2], xr_c[:], SIN).then_inc(s_sc)

        @block.tensor
        def _(pe):
            def emit_tail(j):
                # transpose sc_loc [128,8] -> ps_t2 [8,128] for sending
                pe.wait_ge(s_sc, j + 1)
                if j == 0:
                    pe.wait_ge(d_ld, 48)
                pe.transpose(ps_t2[:], sc_loc[:], eye_sb[:]).then_inc(s_T2)
                # transpose gathered [64,128] -> ps_g [128,64]
                pe.wait_ge(d_out, 16 * (j + 1))
                pe.transpose(
                    ps_g[:], gath_fm[:], eye_sb[0:2 * NBLK, 0:2 * NBLK]
                ).then_inc(s_g)

            emit_tail(0)
            for i in range(n_steps):
                # matvec: 32 accumulating f32r matmuls into ps_mv[0:2,:]
                pe.wait_ge(s_scfull, i + 1)
                if i == 0:
                    pe.wait_ge(d_k, 16)
                for b in range(NBLK):
                    mm = pe.matmul(
                        ps_mv[0:2, :],
                        sc_full[:, 2 * b:2 * b + 2],
                        k_sb[:, LOCAL * b:LOCAL * (b + 1)],
                        start=(b == 0),
                        stop=(b == NBLK - 1),
                    )
                mm.then_inc(s_mv)
                # transpose KsKc [2,512] -> ps_T [128,8] (4x [2,128]->[128,2])
                pe.wait_ge(s_kskc, i + 1)
                for q in range(NSUB):
                    tr = pe.transpose(
                        ps_T[:, 2 * q:2 * q + 2],
                        kskc[:, 128 * q:128 * (q + 1)],
                        eye_sb[0:2, 0:2],
                    )
                tr.then_inc(s_T)
                if i < n_steps - 1:
                    emit_tail(i + 1)

        @block.vector
        def _(dve):
            def emit_reduce():
                # sin arg: xr_s = th - round(th/2pi)*2pi, clamped to +-PI_LO
                dve.tensor_scalar(u1[:], th_sb[:], INV_2PI, MAGIC, AO.mult, AO.add)
                dve.drain()
                dve.tensor_scalar(u1[:], u1[:], MAGIC, None, AO.subtract)
                dve.drain()
                dve.scalar_tensor_tensor(
                    xr_s[:], u1[:], -TWO_PI, th_sb[:], AO.mult, AO.add
                )
                dve.drain()
                dve.tensor_scalar(xr_s[:], xr_s[:], PI_LO, -PI_LO, AO.min, AO.max)
                # cos arg: xr_c = th + pi/2 - round((th+pi/2)/2pi)*2pi
                dve.tensor_scalar(u1[:], th_sb[:], INV_2PI, 0.25, AO.mult, AO.add)
                dve.drain()
                dve.tensor_scalar(u1[:], u1[:], MAGIC, MAGIC, AO.add, AO.subtract)
                dve.drain()
                dve.scalar_tensor_tensor(
                    xr_c[:], u1[:], -TWO_PI, th_sb[:], AO.mult, AO.add
                )
                dve.drain()
                dve.tensor_scalar(
                    xr_c[:], xr_c[:], HALF_PI, PI_LO, AO.add, AO.min
                )
                dve.drain()
                return dve.tensor_scalar(xr_c[:], xr_c[:], -PI_LO, None, AO.max)

            # prologue: item 0
            dve.wait_ge(d_ld, 48)
            emit_reduce().then_inc(s_th)
            dve.wait_ge(s_T2, 1)
            dve.tensor_copy(sc_fm[:], ps_t2[:]).then_inc(s_fm)
            dve.wait_ge(s_g, 1)
            dve.tensor_copy(sc_full[:], ps_g[:]).then_inc(s_scfull)

            for i in range(n_steps):
                # partial KsKc copy out of PSUM
                dve.wait_ge(s_mv, i + 1)
                dve.tensor_copy(kskc[:], ps_mv[0:2, :]).then_inc(s_kskc)
                # coupling + theta update
                dve.wait_ge(s_T, i + 1)
                dve.tensor_tensor(
                    m1[:], sc_loc[:, 1:2 * NSUB:2], ps_T[:, 0:2 * NSUB:2], AO.mult
                )
                dve.tensor_tensor(
                    m2[:], sc_loc[:, 0:2 * NSUB:2], ps_T[:, 1:2 * NSUB:2], AO.mult
                )
                dve.drain()
                dve.tensor_tensor(cpl[:], m1[:], m2[:], AO.subtract)
                dve.drain()
                dve.tensor_tensor(cpl[:], cpl[:], om_sb[:], AO.add)
                dve.drain()
                upd = dve.scalar_tensor_tensor(
                    th_sb[:], cpl[:], DT, th_sb[:], AO.mult, AO.add
                )
                dve.drain()
                if i < n_steps - 1:
                    emit_reduce().then_inc(s_th)
                    # send path: sc_fm <- ps_t2 (item i+1)
                    dve.wait_ge(s_T2, i + 2)
                    dve.wait_ge(d_in, 16 * (i + 1))  # WAR: dma1 item i done
                    dve.tensor_copy(sc_fm[:], ps_t2[:]).then_inc(s_fm)
                    # receive path: sc_full <- ps_g (item i+1)
                    dve.wait_ge(s_g, i + 2)
                    dve.tensor_copy(sc_full[:], ps_g[:]).then_inc(s_scfull)
                else:
                    upd.then_inc(s_th)

    nc.finalize()
    return nc


def prep_inputs(phases, K, omegas):
    """Slice the full inputs into per-core input maps."""
    eye = np.eye(128, dtype=np.float32)
    in_maps = []
    for r in range(NCORES):
        sl = slice(r * LOCAL, (r + 1) * LOCAL)
        in_maps.append({
            "kt": np.ascontiguousarray(K[sl, :].T),
            "th0": np.ascontiguousarray(phases[sl].reshape(NSUB, 128).T),
            "om": np.ascontiguousarray(omegas[sl].reshape(NSUB, 128).T),
            "eye": eye,
        })
    return in_maps


def assemble_output(results):
    return np.concatenate(
        [np.asarray(res["out"]).T.reshape(LOCAL) for res in results]
    ).astype(np.float32)


_NC_CACHE = {}


def kernel(phases, K, omegas):
    from concourse.bass_utils import run_bass_kernel_spmd

    if "nc" not in _NC_CACHE:
        _NC_CACHE["nc"] = build_nc(N_STEPS)
    nc = _NC_CACHE["nc"]
    in_maps = prep_inputs(
        np.asarray(phases, dtype=np.float32),
        np.asarray(K, dtype=np.float32),
        np.asarray(omegas, dtype=np.float32),
    )
    res = run_bass_kernel_spmd(
        nc, in_maps, core_ids=list(range(NCORES)),
        trace=bool(int(os.environ.get("KERNEL_TRACE", "0"))),
    )
    out = assemble_output(res.results)
    if res.exec_time_ns is not None:
        print(f"HW exec time: {res.exec_time_ns} ns")
    return out
